# revision 1
# baseline (speedup 1.0000x reference)
"""Trainium2 Bass kernel for CausalSelfAttention (B=2, T=2048, D=1024, H=16).

Sharding (8 cores): Megatron-style tensor parallel. Core c owns heads
{2c, 2c+1}: column-parallel c_attn (384 of 3072 output features),
full attention for its 2 heads x 2 batches, row-parallel c_proj
(128 of 1024 contraction rows). Host sums the 8 partial outputs and
adds b_proj.

Device algorithm (per core), all matmuls bf16, softmax f32:
  1. qkv^T = Wslice^T @ x^T   -- x^T arrives pre-transposed bf16 from host.
     q^T, k^T stay in SBUF; v^T tiles are PE-transposed (identity matmul)
     into natural [k, d] layout, rows scaled by the key mask, with the
     0/1 key-mask column appended (col 64/129).
  2. Attention in the TRANSPOSED orientation: S^T[k, q] = k^T.T @ q^T
     per 128-row k-tile; exp(S^T - 10) on ACT straight out of PSUM
     (no max-subtraction: logits are O(1), the -10 shift cancels in the
     softmax ratio); causal handled by skipping invalid column ranges +
     an upper-triangular multiplicative mask on diagonal blocks.
  3. PV: out[65, q] = [v_h | mask01].T @ P^T accumulated over k-tiles.
     Row 64 is the softmax denominator (masked keys excluded via the
     zeroed v rows + mask column). rq = qmask / (denom + eps) is
     broadcast across partitions with a ones-matmul and multiplied in.
  4. out = y^T.T @ Wproj_rows -> partial [4096, 1024] f32, host-summed.
"""

import functools

import numpy as np
import ml_dtypes

import concourse.bass as bass
import concourse.mybir as mybir
import concourse.tile as tile
from concourse import bacc
from concourse.bass_utils import run_bass_kernel_spmd
from concourse.masks import make_upper_triangular, make_identity

BF16 = mybir.dt.bfloat16
F32 = mybir.dt.float32
AF = mybir.ActivationFunctionType
OP = mybir.AluOpType

B, T, D, NH = 2, 2048, 1024, 16
DH = 64                  # head dim
HPC = 2                  # heads per core
NCORES = 8
TT = B * T               # 4096 total tokens
P = 128
KC = D // P              # 8 contraction tiles for qkv
M3 = 3 * HPC * DH // P   # 3 feature tiles (q,k,v each 128 wide)
SPAN = 512               # q-span processed per softmax pass
NSP = T // SPAN          # 4 spans per batch
NKT = T // P             # 16 k-tiles per batch
QSCALE = 1.0 / np.sqrt(DH)
ESHIFT = -10.0           # constant exp shift; cancels in softmax ratio


def build(debug_outs=False):
    nc = bacc.Bacc(None)

    xT = nc.dram_tensor("xT", [D, TT], BF16, kind="ExternalInput")
    wqkv = nc.dram_tensor("wqkv", [KC, P, 3 * P], BF16, kind="ExternalInput")
    bqkv = nc.dram_tensor("bqkv", [P, 3], F32, kind="ExternalInput")
    wproj = nc.dram_tensor("wproj", [P, D], BF16, kind="ExternalInput")
    mrowinv = nc.dram_tensor("mrowinv", [1, TT], F32, kind="ExternalInput")
    mcol = nc.dram_tensor("mcol", [B, P, NKT], F32, kind="ExternalInput")
    out = nc.dram_tensor("out", [TT, D], BF16, kind="ExternalOutput")

    with tile.TileContext(nc) as tc:
        with (
            tc.tile_pool(name="singles", bufs=1) as singles,
            tc.tile_pool(name="stage", bufs=2) as stage,
            tc.tile_pool(name="pt", bufs=4) as ptp,
            tc.tile_pool(name="rows", bufs=2) as rows,
            tc.tile_pool(name="outs", bufs=3) as outs,
            tc.tile_pool(name="ps", bufs=2, space="PSUM") as ps,
            tc.tile_pool(name="ps2", bufs=2, space="PSUM") as ps2,
        ):
            # ---- constants / weights (small DMAs first: casts block on them) ----
            wqkv_sb = singles.tile([P, KC, 3 * P], BF16)
            nc.sync.dma_start(out=wqkv_sb, in_=wqkv.rearrange("k p m -> p k m"))
            bqkv_sb = singles.tile([P, 3], F32)
            nc.sync.dma_start(out=bqkv_sb, in_=bqkv[:, :])
            wproj_sb = singles.tile([P, D], BF16)
            nc.sync.dma_start(out=wproj_sb, in_=wproj[:, :])
            mrowinv_sb = singles.tile([1, TT], F32)
            nc.sync.dma_start(out=mrowinv_sb, in_=mrowinv[:, :])
            mcol_sb = singles.tile([P, B, NKT], F32)
            nc.sync.dma_start(out=mcol_sb, in_=mcol.rearrange("b p o -> p b o"))
            xT_sb = singles.tile([P, KC, TT], BF16)
            for n2 in range(TT // 1024):
                for k in range(KC):
                    tsl = slice(n2 * 1024, (n2 + 1) * 1024)
                    nc.sync.dma_start(out=xT_sb[:, k, tsl],
                                      in_=xT[k * P:(k + 1) * P, tsl])

            eshift_sb = singles.tile([P, 1], F32)
            nc.vector.memset(eshift_sb, ESHIFT)
            ut_sb = singles.tile([P, P], BF16)  # keep q >= k
            make_upper_triangular(nc, ut_sb, val=1.0, diag=True)
            ident = singles.tile([P, P], BF16)
            make_identity(nc, ident)

            qT_sb = singles.tile([P, TT], BF16)   # rows: h0 d0..63 | h1 d0..63
            kT_sb = singles.tile([P, TT], BF16)
            yT_sb = singles.tile([P, TT], BF16)
            v_nat = singles.tile([P, NKT * B, 2 * DH + 2], BF16)
            VW = 2 * DH + 2  # 130

            # ---- phase 1: qkv^T = W^T @ x^T ----
            # groups of [128 feat, 1024 t]; n2-outer so attention can start early
            for n2 in range(TT // 1024):
                for m in range(M3):
                    pq = ps.tile([P, 1024], F32, tag="big")
                    for k in range(KC):
                        for h2 in range(2):
                            nc.tensor.matmul(
                                pq[:, h2 * 512:(h2 + 1) * 512],
                                wqkv_sb[:, k, m * P:(m + 1) * P],
                                xT_sb[:, k, n2 * 1024 + h2 * 512: n2 * 1024 + (h2 + 1) * 512],
                                start=(k == 0), stop=(k == KC - 1),
                            )
                    tcols = slice(n2 * 1024, (n2 + 1) * 1024)
                    if m == 0:
                        nc.scalar.activation(
                            qT_sb[:, tcols], pq[:], AF.Identity,
                            bias=bqkv_sb[:, 0:1], scale=QSCALE)
                    elif m == 1:
                        nc.scalar.activation(
                            kT_sb[:, tcols], pq[:], AF.Identity,
                            bias=bqkv_sb[:, 1:2], scale=1.0)
                    else:
                        vst = stage.tile([P, 1024], BF16, tag="vst")
                        nc.scalar.activation(
                            vst[:], pq[:], AF.Identity,
                            bias=bqkv_sb[:, 2:3], scale=1.0)
                        # phase 2: v natural [k, d] via PE transpose + key mask
                        for jj in range(1024 // P):
                            j32 = n2 * 8 + jj
                            b, o = divmod(j32, NKT)
                            vtp = ps2.tile([P, P], BF16, tag="vtp")
                            nc.tensor.transpose(
                                vtp[:], vst[:, jj * P:(jj + 1) * P], ident[:])
                            nc.vector.tensor_scalar_mul(
                                v_nat[:, j32, 0:DH], vtp[:, 0:DH],
                                mcol_sb[:, b, o:o + 1])
                            nc.vector.tensor_scalar_mul(
                                v_nat[:, j32, DH + 1:2 * DH + 1],
                                vtp[:, DH:2 * DH], mcol_sb[:, b, o:o + 1])
                            nc.vector.tensor_copy(out=v_nat[:, j32, DH:DH + 1],
                                                  in_=mcol_sb[:, b, o:o + 1])
                            nc.vector.tensor_copy(out=v_nat[:, j32, VW - 1:VW],
                                                  in_=mcol_sb[:, b, o:o + 1])

            # ---- phase 3: attention, transposed orientation ----
            def emit_proj(tt):
                ob = outs.tile([P, D], BF16, tag="ob")
                for half in range(2):
                    po = ps2.tile([P, 512], F32, tag="vtp", name="po")
                    nc.tensor.matmul(
                        po[:],
                        yT_sb[:, tt * P:(tt + 1) * P],
                        wproj_sb[:, half * 512:(half + 1) * 512],
                        start=True, stop=True,
                    )
                    nc.vector.tensor_copy(out=ob[:, half * 512:(half + 1) * 512],
                                          in_=po[:])
                nc.sync.dma_start(out=out[tt * P:(tt + 1) * P, :], in_=ob)

            for b in range(B):
                for s in range(NSP):
                    qg = b * T + s * SPAN          # global q col base
                    njs = 4 * s + 4                # k-tiles for this span
                    pvs = [ps.tile([DH + 1, SPAN], F32, tag="pv", name=f"pv{_h}")
                           for _h in range(HPC)]
                    for jj in range(0, njs, 2):
                        sts, pts, offs = [], [], []
                        for h in range(HPC):
                            sts.append(ps.tile([P, 1024], F32, tag="big",
                                               name=f"st{h}"))
                            pts.append(ptp.tile([P, 1024], BF16, tag="pt",
                                                name=f"pt{h}"))
                        for dj in range(2):
                            j = jj + dj
                            off = max(0, j - 4 * s) * P
                            offs.append(off)
                            kb = b * T + j * P
                            for h in range(HPC):
                                hb = h * DH
                                nc.tensor.matmul(
                                    sts[h][:, dj * 512 + off:(dj + 1) * 512],
                                    kT_sb[hb:hb + DH, kb:kb + P],
                                    qT_sb[hb:hb + DH, qg + off:qg + SPAN],
                                    start=True, stop=True,
                                )
                        for h in range(HPC):
                            if offs[1] == 0:
                                nc.scalar.activation(
                                    pts[h][:], sts[h][:], AF.Exp, bias=eshift_sb[:])
                            else:
                                for dj in range(2):
                                    csl = slice(dj * 512 + offs[dj], (dj + 1) * 512)
                                    nc.scalar.activation(
                                        pts[h][:, csl], sts[h][:, csl],
                                        AF.Exp, bias=eshift_sb[:])
                        for dj in range(2):
                            j = jj + dj
                            off = offs[dj]
                            for h in range(HPC):
                                if j >= 4 * s:  # diagonal block: keep q >= k
                                    dsl = slice(dj * 512 + off, dj * 512 + off + P)
                                    nc.vector.tensor_tensor(
                                        pts[h][:, dsl], pts[h][:, dsl], ut_sb[:],
                                        OP.mult)
                                vc0 = h * (DH + 1)
                                nc.tensor.matmul(
                                    pvs[h][:, off:SPAN],
                                    v_nat[:, b * NKT + j, vc0:vc0 + DH + 1],
                                    pts[h][:, dj * 512 + off:(dj + 1) * 512],
                                    start=(j == 0), stop=(j == njs - 1),
                                )
                    for h in range(HPC):
                        den = rows.tile([1, SPAN], F32, tag="den")
                        nc.vector.tensor_tensor(
                            den, pvs[h][DH:DH + 1, :],
                            mrowinv_sb[0:1, qg:qg + SPAN], OP.add)
                        rq = rows.tile([1, SPAN], F32, tag="rq")
                        nc.vector.reciprocal_approx_fast(out=rq, in_=den)
                        bc_sb = rows.tile([DH, SPAN], F32, tag="bcs")
                        nc.gpsimd.partition_broadcast(bc_sb[:], rq[:])
                        hb = h * DH
                        nc.vector.tensor_tensor(
                            yT_sb[hb:hb + DH, qg:qg + SPAN],
                            pvs[h][0:DH, :], bc_sb[:], OP.mult)
                    for tt in range(qg // P, (qg + SPAN) // P):
                        emit_proj(tt)


            if debug_outs:
                d_ut = nc.dram_tensor("d_ut", [P, P], BF16, kind="ExternalOutput")
                d_qT = nc.dram_tensor("d_qT", [P, TT], BF16, kind="ExternalOutput")
                d_kT = nc.dram_tensor("d_kT", [P, TT], BF16, kind="ExternalOutput")
                d_yT = nc.dram_tensor("d_yT", [P, TT], BF16, kind="ExternalOutput")
                d_vn = nc.dram_tensor("d_vn", [P, NKT * B * VW], BF16,
                                      kind="ExternalOutput")
                nc.sync.dma_start(out=d_ut[:, :], in_=ut_sb)
                nc.sync.dma_start(out=d_qT[:, :], in_=qT_sb)
                nc.sync.dma_start(out=d_kT[:, :], in_=kT_sb)
                nc.sync.dma_start(out=d_yT[:, :], in_=yT_sb)
                nc.sync.dma_start(
                    out=d_vn.rearrange("p (j w) -> p j w", w=VW), in_=v_nat)


    nc.finalize()
    return nc


@functools.lru_cache(maxsize=1)
def _built():
    return build()


def _prep_core(c, x, attention_mask, W_attn, b_attn, W_proj):
    bf = ml_dtypes.bfloat16
    q0 = c * HPC * DH
    qs = slice(q0, q0 + P)
    ks = slice(D + q0, D + q0 + P)
    vs = slice(2 * D + q0, 2 * D + q0 + P)
    wsl = np.concatenate(
        [W_attn[:, qs], W_attn[:, ks], W_attn[:, vs]], axis=1)  # [1024, 384]
    bq = b_attn[qs] * QSCALE
    return {
        "wqkv": np.ascontiguousarray(wsl.reshape(KC, P, 3 * P)).astype(bf),
        "bqkv": np.ascontiguousarray(
            np.stack([bq, b_attn[ks], b_attn[vs]], axis=1)).astype(np.float32),
        "wproj": np.ascontiguousarray(W_proj[qs, :]).astype(bf),
    }


def build_in_maps(x, attention_mask, W_attn, b_attn, W_proj):
    bf = ml_dtypes.bfloat16
    x = np.asarray(x, dtype=np.float32)
    attention_mask = np.asarray(attention_mask)
    W_attn = np.asarray(W_attn, dtype=np.float32)
    b_attn = np.asarray(b_attn, dtype=np.float32)
    W_proj = np.asarray(W_proj, dtype=np.float32)

    xT = np.ascontiguousarray(x.reshape(TT, D).T).astype(bf)
    maskf = attention_mask.astype(np.float32)
    mrowinv = np.ascontiguousarray(
        ((1.0 - maskf) * 1e30 + 1e-20).reshape(1, TT)).astype(np.float32)
    mcol = np.ascontiguousarray(
        maskf.reshape(B, NKT, P).transpose(0, 2, 1)).astype(np.float32)  # [B, P, NKT]

    in_maps = []
    for c in range(NCORES):
        m = _prep_core(c, x, attention_mask, W_attn, b_attn, W_proj)
        m["xT"] = xT
        m["mrowinv"] = mrowinv
        m["mcol"] = mcol
        in_maps.append(m)
    return in_maps


def kernel(x, attention_mask, W_attn, b_attn, W_proj, b_proj):
    b_proj = np.asarray(b_proj, dtype=np.float32)
    nc = _built()
    in_maps = build_in_maps(x, attention_mask, W_attn, b_attn, W_proj)
    res = run_bass_kernel_spmd(nc, in_maps, core_ids=list(range(NCORES)))
    acc = np.zeros((TT, D), dtype=np.float32)
    for c in range(NCORES):
        acc += res.results[c]["out"].astype(np.float32)
    acc += b_proj[None, :]
    return acc.reshape(B, T, D)



# revision 31
# speedup vs baseline: 25190.9443x; 25190.9443x over previous
"""Trainium2 Bass kernel for CausalSelfAttention (B=2, T=2048, D=1024, H=16).

Sharding (8 cores): Megatron-style tensor parallel. Core c owns heads
{2c, 2c+1}: column-parallel c_attn (384 of 3072 output features),
full attention for its 2 heads x 2 batches, row-parallel c_proj
(128 of 1024 contraction rows). Host sums the 8 partial outputs and
adds b_proj (+ b_v @ W_proj, folded host-side since the v bias is a
per-feature constant that commutes through softmax-weighted averaging).

Device algorithm (per core):
  1. q^T, k^T = W^T @ x^T in bf16 (fp8 here costs ~3% output error at
     peaked softmax rows). v is computed DIRECTLY in natural
     [token, feat] layout by using the x^T tile as the matmul stationary
     (no PE transpose); its epilogue scales by mask*SV per token (key
     mask folded in) and writes bf16 v_natb; a Pool-engine downcast
     produces the fp8e4 copy v_nat used by DoubleRow PV.
  2. Attention in the transposed orientation: S^T[k, q] = k^T.T @ q^T
     per 128-row k-tile; the causal mask is ADDED into the S^T PSUM by
     an identity-stationary matmul (-30 above the diagonal), so exp
     output is already causal and no elementwise mask op exists on any
     engine. exp on ACT straight out of PSUM (no max-subtraction:
     logits are O(1) and the softmax ratio is shift-invariant).
     Non-diagonal k-tile pairs exp to fp8e5 (wide exponent range: no
     overflow/subnormal issues) and feed fp8 DoubleRow PV matmuls
     (2 k-tiles contracted per instruction); diagonal tiles exp to
     bf16 and feed bf16-moving x bf16-stationary PV (full precision
     where few keys dominate). Per-head v blocks are padded to 80
     columns (DoubleRow weight loads need M % 16 == 0).
  3. PV accumulates [65, q] per head: row 64 is the softmax denominator
     (0/1 mask column x P, so masked keys drop out of both numerator
     and denominator). pvs PSUM is drained immediately (unscaled y and
     denominator) to free the ring for the next span; then
     rq = qmask/(den + eps) is partition-broadcast (gpsimd) and
     multiplied in on DVE.
  4. out = y^T.T @ Wproj_rows -> partial [4096, 1024] bf16, PSUM->SBUF
     casts split 2:1 across DVE and ACT, host-summed in f32.

Scheduling: per-1024-token groups interleave QKV(0) QKV(1) ATT(0)
QKV(2) ATT(1) QKV(3) ATT(2) ATT(3). Within attention, PV is emitted one
pair late and QKV/proj work is kept in a filler queue pumped right
after each exp, so the PE always has ready work during exp waits (PE
DVFS: idle gaps drop the clock to mid-pstate for ~3us).
"""
import functools

import numpy as np
import ml_dtypes

import concourse.bass as bass
import concourse.mybir as mybir
import concourse.tile as tile
from concourse import bacc
from concourse.bass_utils import run_bass_kernel_spmd
from concourse.masks import make_upper_triangular, make_identity

BF16 = mybir.dt.bfloat16
F32 = mybir.dt.float32
FP8 = mybir.dt.float8e4
FP8E5 = mybir.dt.float8e5
AF = mybir.ActivationFunctionType
OP = mybir.AluOpType
PM = mybir.MatmulPerfMode

B, T, D, NH = 2, 2048, 1024, 16
DH = 64                  # head dim
HPC = 2                  # heads per core
NCORES = 8
TT = B * T               # 4096 total tokens
P = 128
KC = D // P              # 8 contraction tiles for qkv
SPAN = 512               # q-span processed per softmax pass
NKT = T // P             # 16 k-tiles per batch
NTOK = TT // P           # 32 token tiles
QSCALE = 1.0 / np.sqrt(DH)
SV = 8.0                 # fp8 v pre-scale (folded back out after PV)
VS = 80                  # per-head v_nat stride (DoubleRow needs M%16==0)
VW = 2 * VS              # v_nat width: v_h0|mask|pad | v_h1|mask|pad
VB = 66                  # per-head v_natb (bf16) stride: v|mask|pad


def build(debug_outs=False):
    nc = bacc.Bacc(None)

    xTb = nc.dram_tensor("xTb", [D, TT], BF16, kind="ExternalInput")
    wqkvb = nc.dram_tensor("wqkvb", [KC, P, 3 * P], BF16, kind="ExternalInput")
    bqk = nc.dram_tensor("bqk", [P, 2], F32, kind="ExternalInput")
    wproj = nc.dram_tensor("wproj", [P, D], BF16, kind="ExternalInput")
    mrowinv = nc.dram_tensor("mrowinv", [1, TT], F32, kind="ExternalInput")
    msv = nc.dram_tensor("msv", [P, NTOK], F32, kind="ExternalInput")
    mcol8 = nc.dram_tensor("mcol8", [P, NTOK, 1], FP8, kind="ExternalInput")
    out = nc.dram_tensor("out", [TT, D], BF16, kind="ExternalOutput")

    with tile.TileContext(nc) as tc:
        with (
            tc.tile_pool(name="singles", bufs=1) as singles,
            tc.tile_pool(name="stage", bufs=4) as stage,
            tc.tile_pool(name="rows", bufs=2) as rows,
            tc.tile_pool(name="outs", bufs=3) as outs,
            tc.tile_pool(name="ps", bufs=2, space="PSUM") as ps,
            tc.tile_pool(name="psv", bufs=2, space="PSUM") as psv,
            tc.tile_pool(name="ps2", bufs=2, space="PSUM") as ps2,
        ):
            # ---- constants / weights (small DMAs first) ----
            wqkvb_sb = singles.tile([P, KC, 3 * P], BF16)
            for m in range(3):  # split q|k|v so q-weights land first
                nc.sync.dma_start(
                    out=wqkvb_sb[:, :, m * P:(m + 1) * P],
                    in_=wqkvb.rearrange("k p m -> p k m")[:, :, m * P:(m + 1) * P])
            bqk_sb = singles.tile([P, 2], F32)
            nc.sync.dma_start(out=bqk_sb, in_=bqk[:, :])
            wproj_sb = singles.tile([P, D], BF16)
            nc.sync.dma_start(out=wproj_sb, in_=wproj[:, :])
            mrowinv_sb = singles.tile([1, TT], F32)
            nc.sync.dma_start(out=mrowinv_sb, in_=mrowinv[:, :])
            msv_sb = singles.tile([P, NTOK], F32)
            nc.sync.dma_start(out=msv_sb, in_=msv[:, :])
            mcol8_sb = singles.tile([P, NTOK, 1], FP8)
            nc.sync.dma_start(out=mcol8_sb, in_=mcol8[:, :, :])

            # 512-token stripes so the first QKV chunk's inputs land fast
            xTb_sb = singles.tile([P, KC, TT], BF16)
            for g in range(4):
                for half in range(2):
                    for k in range(KC):
                        tsl = slice(g * 1024 + half * 512,
                                    g * 1024 + (half + 1) * 512)
                        nc.sync.dma_start(out=xTb_sb[:, k, tsl],
                                          in_=xTb[k * P:(k + 1) * P, tsl])

            # additive causal masks: accumulated into S^T PSUM via an
            # identity-stationary matmul, so exp output is already causal
            # (no elementwise mask op on any engine).
            ut_sb = singles.tile([P, P], BF16)
            make_upper_triangular(nc, ut_sb, val=1.0, diag=True)
            ident = singles.tile([P, P], BF16)
            make_identity(nc, ident)
            negm1 = singles.tile([P, P], BF16)   # -30 where q < k (diag block)
            nc.vector.tensor_scalar(out=negm1, in0=ut_sb, scalar1=30.0,
                                    scalar2=-30.0, op0=OP.mult, op1=OP.add)
            negm2 = singles.tile([P, 2 * P], BF16)  # dj=1: full block + diag
            nc.vector.memset(negm2[:, 0:P], -30.0)
            nc.vector.tensor_copy(out=negm2[:, P:2 * P], in_=negm1[:])

            qT_sb = singles.tile([P, TT], BF16)   # rows: h0 d0..63 | h1 d0..63
            kT_sb = singles.tile([P, TT], BF16)
            yT_sb = singles.tile([P, TT], BF16)
            v_nat = singles.tile([P, NTOK, VW], FP8)
            v_natb = singles.tile([P, NTOK, 2 * VB], BF16)  # bf16 v for diag PV
            # mask columns once; zero the fp8 pad columns
            nc.vector.tensor_copy(out=v_nat[:, :, DH:DH + 1], in_=mcol8_sb)
            nc.vector.tensor_copy(out=v_nat[:, :, VS + DH:VS + DH + 1],
                                  in_=mcol8_sb)
            nc.vector.tensor_copy(out=v_natb[:, :, DH:DH + 1], in_=mcol8_sb)
            nc.vector.tensor_copy(out=v_natb[:, :, VB + DH:VB + DH + 1],
                                  in_=mcol8_sb)
            nc.vector.memset(v_nat[:, :, DH + 1:VS], 0.0)
            nc.vector.memset(v_nat[:, :, VS + DH + 1:VW], 0.0)

            # ---- QKV chunks (emitted as PE gap-fillers) ----
            def emit_qk_chunk(g, m, h2):
                tsl = slice(g * 1024 + h2 * 512, g * 1024 + (h2 + 1) * 512)
                pq = ps2.tile([P, 512], F32, tag="po", name="pq")
                for k in range(KC):
                    nc.tensor.matmul(
                        pq[:],
                        wqkvb_sb[:, k, m * P:(m + 1) * P],
                        xTb_sb[:, k, tsl],
                        start=(k == 0), stop=(k == KC - 1),
                    )
                dst = qT_sb if m == 0 else kT_sb
                nc.vector.tensor_scalar(
                    out=dst[:, tsl], in0=pq[:],
                    scalar1=QSCALE if m == 0 else 1.0,
                    scalar2=bqk_sb[:, m:m + 1],
                    op0=OP.mult, op1=OP.add)

            def emit_v_chunk(g, gg):
                # v direct in natural [token, feat] layout
                vps = ps2.tile([P, 512], F32, tag="po", name="vps")
                for i in range(4):
                    j32 = g * 8 + gg * 4 + i
                    tsl = slice(j32 * P, (j32 + 1) * P)
                    for k in range(KC):
                        nc.tensor.matmul(
                            vps[:, i * P:(i + 1) * P],
                            xTb_sb[:, k, tsl],
                            wqkvb_sb[:, k, 2 * P:3 * P],
                            start=(k == 0), stop=(k == KC - 1),
                        )
                for i in range(4):
                    j32 = g * 8 + gg * 4 + i
                    vbview = v_natb[:, j32, :].rearrange(
                        "p (h w) -> p h w", w=VB)[:, :, 0:DH]
                    pview = vps[:, i * P:(i + 1) * P].rearrange(
                        "p (h w) -> p h w", w=DH)
                    nc.vector.tensor_scalar_mul(
                        vbview, pview, msv_sb[:, j32:j32 + 1])
                    vview = v_nat[:, j32, :].rearrange(
                        "p (h w) -> p h w", w=VS)[:, :, 0:DH]
                    nc.gpsimd.tensor_copy(out=vview, in_=vbview)

            def emit_qkv(g):
                for m in range(2):
                    for h2 in range(2):
                        emit_qk_chunk(g, m, h2)
                for gg in range(2):
                    emit_v_chunk(g, gg)

            fillers = []

            def pump():
                if fillers:
                    fillers.pop(0)()

            # ---- proj for one 128-token tile ----
            proj_n = [0]

            def emit_proj(tt):
                ob = outs.tile([P, D], BF16, tag="ob")
                for half in range(2):
                    po = ps2.tile([P, 512], F32, tag="po", name="po")
                    nc.tensor.matmul(
                        po[:],
                        yT_sb[:, tt * P:(tt + 1) * P],
                        wproj_sb[:, half * 512:(half + 1) * 512],
                        start=True, stop=True,
                    )
                    dst = ob[:, half * 512:(half + 1) * 512]
                    if proj_n[0] % 3 == 2:
                        nc.scalar.activation(dst, po[:], AF.Copy, bias=0.0)
                    else:
                        nc.vector.tensor_copy(out=dst, in_=po[:])
                    proj_n[0] += 1
                nc.sync.dma_start(out=out[tt * P:(tt + 1) * P, :], in_=ob)

            # ---- attention for one group's two spans ----
            def emit_att(g):
                b = g // 2
                for s in (2 * (g % 2), 2 * (g % 2) + 1):
                    qg = b * T + s * SPAN
                    njs = 4 * s + 4
                    pvs = [psv.tile([VS, SPAN], F32, tag="pv", name=f"pv{h}")
                           for h in range(HPC)]
                    started = [False, False]
                    pending = [None]

                    def flush():
                        if pending[0] is not None:
                            pending[0]()
                            pending[0] = None

                    # non-diagonal pairs: fp8 DoubleRow path
                    for jj in range(0, 4 * s, 2):
                        flush()
                        sts = [ps.tile([P, 2, 512], F32, tag="big",
                                       name=f"st{h}") for h in range(HPC)]
                        pts = [stage.tile([P, 2, 512], FP8E5, tag="pt8",
                                          name=f"pt{h}") for h in range(HPC)]
                        for dj in range(2):
                            kb = b * T + (jj + dj) * P
                            for h in range(HPC):
                                hb = h * DH
                                nc.tensor.matmul(
                                    sts[h][:, dj, :],
                                    kT_sb[hb:hb + DH, kb:kb + P],
                                    qT_sb[hb:hb + DH, qg:qg + SPAN],
                                    start=True, stop=True,
                                )
                        for h in range(HPC):
                            nc.scalar.activation(pts[h][:, :, :],
                                                 sts[h][:, :, :], AF.Exp)
                        pump()

                        def pv_nd(jj=jj, pts=pts, st0=started[0]):
                            for h in range(HPC):
                                nc.tensor.matmul(
                                    pvs[h][:, :],
                                    v_nat[:, b * NKT + jj:b * NKT + jj + 2,
                                          h * VS:(h + 1) * VS],
                                    pts[h][:, :, :],
                                    start=not st0, stop=False,
                                    perf_mode=PM.DoubleRow,
                                    skip_group_check=True,
                                )
                        pending[0] = pv_nd
                        started = [True, True]
                    # diagonal pairs: bf16 path, causal via additive PE mask
                    for jj in (4 * s, 4 * s + 2):
                        flush()
                        off0 = (jj - 4 * s) * P
                        sts = [ps.tile([P, 2, 512], F32, tag="big",
                                       name=f"st{h}") for h in range(HPC)]
                        ptd = [stage.tile([P, 2, 512], BF16, tag="ptd",
                                          name=f"pd{h}") for h in range(HPC)]
                        for dj in range(2):
                            # dj=1 also starts at off0 so the merged exp never
                            # reads unwritten PSUM; the additive mask sends
                            # above-diagonal scores to -30.
                            kb = b * T + (jj + dj) * P
                            for h in range(HPC):
                                hb = h * DH
                                nc.tensor.matmul(
                                    sts[h][:, dj, off0:SPAN],
                                    kT_sb[hb:hb + DH, kb:kb + P],
                                    qT_sb[hb:hb + DH, qg + off0:qg + SPAN],
                                    start=True, stop=False,
                                )
                                mw = P if dj == 0 else 2 * P
                                nc.tensor.matmul(
                                    sts[h][:, dj, off0:off0 + mw],
                                    ident[:],
                                    (negm1 if dj == 0 else negm2)[:, 0:mw],
                                    start=False, stop=True,
                                    skip_group_check=True,
                                )
                        for h in range(HPC):
                            nc.scalar.activation(ptd[h][:, :, off0:SPAN],
                                                 sts[h][:, :, off0:SPAN],
                                                 AF.Exp)
                        pump()

                        def pv_d(jj=jj, ptd=ptd, off0=off0, st0=started[0]):
                            for h in range(HPC):
                                for dj in range(2):
                                    j = jj + dj
                                    off = off0 + dj * P
                                    nc.tensor.matmul(
                                        pvs[h][0:DH + 1, off:SPAN],
                                        v_natb[:, b * NKT + j,
                                               h * VB:h * VB + DH + 1],
                                        ptd[h][:, dj, off:SPAN],
                                        start=(not st0) and dj == 0,
                                        stop=(j == njs - 1),
                                        skip_group_check=True,
                                    )
                        pending[0] = pv_d
                        started = [True, True]
                    flush()
                    # drain pvs fast (frees the psum ring for the next span),
                    # then normalize yT in place with all-SBUF bf16 ops
                    dens = []
                    yraw = rows.tile([DH, HPC, SPAN], BF16, tag="yraw")
                    for h in range(HPC):
                        nc.vector.tensor_scalar_mul(
                            yraw[:, h, :], pvs[h][0:DH, :], 1.0 / SV)
                        den = rows.tile([1, SPAN], F32, tag=f"den{h}")
                        nc.vector.tensor_tensor(
                            den, pvs[h][DH:DH + 1, :],
                            mrowinv_sb[0:1, qg:qg + SPAN], OP.add)
                        dens.append(den)
                    for h in range(HPC):
                        rq = rows.tile([1, SPAN], F32, tag="rq")
                        nc.vector.reciprocal_approx_fast(out=rq, in_=dens[h])
                        bc_sb = rows.tile([DH, SPAN], F32, tag="bcs")
                        nc.gpsimd.partition_broadcast(bc_sb[:], rq[:])
                        hb = h * DH
                        nc.vector.tensor_tensor(
                            yT_sb[hb:hb + DH, qg:qg + SPAN],
                            yraw[:, h, :], bc_sb[:], OP.mult)
                    for tt in range(qg // P, (qg + SPAN) // P):
                        fillers.append(lambda tt=tt: emit_proj(tt))

            emit_qkv(0)
            emit_qkv(1)
            fillers.extend([
                (lambda m=m, h2=h2: emit_qk_chunk(2, m, h2))
                for m in range(2) for h2 in range(2)])
            fillers.extend([(lambda gg=gg: emit_v_chunk(2, gg))
                            for gg in range(2)])
            emit_att(0)
            fillers.extend([
                (lambda m=m, h2=h2: emit_qk_chunk(3, m, h2))
                for m in range(2) for h2 in range(2)])
            fillers.extend([(lambda gg=gg: emit_v_chunk(3, gg))
                            for gg in range(2)])
            emit_att(1)
            emit_att(2)
            emit_att(3)
            while fillers:
                pump()

            if debug_outs:
                d_qT = nc.dram_tensor("d_qT", [P, TT], BF16, kind="ExternalOutput")
                d_kT = nc.dram_tensor("d_kT", [P, TT], BF16, kind="ExternalOutput")
                d_yT = nc.dram_tensor("d_yT", [P, TT], BF16, kind="ExternalOutput")
                d_vn = nc.dram_tensor("d_vn", [P, NTOK * VW], FP8,
                                      kind="ExternalOutput")
                nc.sync.dma_start(out=d_qT[:, :], in_=qT_sb)
                nc.sync.dma_start(out=d_kT[:, :], in_=kT_sb)
                nc.sync.dma_start(out=d_yT[:, :], in_=yT_sb)
                nc.sync.dma_start(
                    out=d_vn.rearrange("p (j w) -> p j w", w=VW), in_=v_nat)

    nc.finalize()
    return nc


@functools.lru_cache(maxsize=2)
def _built(debug_outs=False):
    return build(debug_outs)


def _prep_core(c, W_attn, b_attn, W_proj):
    bf = ml_dtypes.bfloat16
    q0 = c * HPC * DH
    qs = slice(q0, q0 + P)
    ks = slice(D + q0, D + q0 + P)
    vs = slice(2 * D + q0, 2 * D + q0 + P)
    wsl = np.concatenate(
        [W_attn[:, qs], W_attn[:, ks], W_attn[:, vs]], axis=1)  # [1024, 384]
    return {
        "wqkvb": np.ascontiguousarray(wsl.reshape(KC, P, 3 * P)).astype(bf),
        "bqk": np.ascontiguousarray(np.stack(
            [b_attn[qs] * QSCALE, b_attn[ks]], axis=1)).astype(np.float32),
        "wproj": np.ascontiguousarray(W_proj[qs, :]).astype(bf),
    }


def build_in_maps(x, attention_mask, W_attn, b_attn, W_proj):
    e4 = ml_dtypes.float8_e4m3
    x = np.asarray(x, dtype=np.float32)
    attention_mask = np.asarray(attention_mask)
    W_attn = np.asarray(W_attn, dtype=np.float32)
    b_attn = np.asarray(b_attn, dtype=np.float32)
    W_proj = np.asarray(W_proj, dtype=np.float32)

    xT = np.ascontiguousarray(x.reshape(TT, D).T)
    xTb = xT.astype(ml_dtypes.bfloat16)
    maskf = attention_mask.astype(np.float32).reshape(TT)
    mrowinv = np.ascontiguousarray(
        ((1.0 - maskf) * 1e30 + 1e-20).reshape(1, TT)).astype(np.float32)
    mtok = np.ascontiguousarray(maskf.reshape(NTOK, P).T)  # [P, 32]
    mcol8 = mtok.astype(e4).reshape(P, NTOK, 1)

    msv = np.ascontiguousarray(mtok * SV).astype(np.float32)
    in_maps = []
    for c in range(NCORES):
        m = _prep_core(c, W_attn, b_attn, W_proj)
        m["xTb"] = xTb
        m["mrowinv"] = mrowinv
        m["msv"] = msv
        m["mcol8"] = mcol8
        in_maps.append(m)
    return in_maps


def kernel(x, attention_mask, W_attn, b_attn, W_proj, b_proj):
    b_proj = np.asarray(b_proj, dtype=np.float32)
    b_attn_np = np.asarray(b_attn, dtype=np.float32)
    W_proj_np = np.asarray(W_proj, dtype=np.float32)
    nc = _built()
    in_maps = build_in_maps(x, attention_mask, W_attn, b_attn, W_proj)
    res = run_bass_kernel_spmd(nc, in_maps, core_ids=list(range(NCORES)))
    acc = np.zeros((TT, D), dtype=np.float32)
    for c in range(NCORES):
        acc += res.results[c]["out"].astype(np.float32)
    acc += (b_proj + b_attn_np[2 * D:] @ W_proj_np)[None, :]
    return acc.reshape(B, T, D)
